# revision 33
# baseline (speedup 1.0000x reference)
"""Grouped-experts SwiGLU MoE kernel for Trainium2 (8 NeuronCores).

Problem: x [8192, 2048] f32, 8 experts with w1/w3 [8, 1408, 2048] and
w2 [8, 2048, 1408]; tokens are expert-contiguous with a per-expert count
vector. out[t] = (silu(x_t @ w1_e.T) * (x_t @ w3_e.T)) @ w2_e.T for the
expert e owning token t.

Sharding: pure expert parallelism. Core e receives expert e's 1024-token
tile (dynamic-slice semantics of the reference) plus expert e's weights,
and computes the full SwiGLU MLP for that tile. No collectives.

Performance design (PE roofline = 540,672 moving rows = 225.3 us at
2.4 GHz, 1 cycle/row; CoreSim predicts 235.7 us, PE 96% busy):
  - matmul operands are bf16: same 1 cycle/row as float32r on the PE,
    but half the HBM traffic and SBUF footprint. End-to-end error vs the
    f32 reference is ~4.5e-3 (fp32 PSUM accumulation), well inside the
    2e-2 gate. All 22 w1/w3 tiles + all 4 w2 tiles stay RESIDENT in SBUF
    (~200 KiB/partition total), so every weight byte is DMAed once.
  - contraction dims (D stage 1, H stage 2) live on SBUF partitions; all
    tensors are packed host-side as [p, ktile, free] so every DMA is a
    contiguous partition-row load and no on-device transposes exist.
  - consecutive matmuls share each stationary tile (both token-blocks
    per w1/w3 tile in stage 1, all four dim-blocks per h-tile in stage
    2): PE weight reloads (~29 ns each on HW, unmodeled in the cost
    model) drop from 1067 to ~450 — measured ~14-40 us faster on HW
    than the chain-per-psum ordering, despite identical row count.
  - the PE executes (nearly) in program order, so the single SP-queue
    DMA stream is sequenced to feed exactly the compute order: half of
    w1[ht0], x k-tiles 0-1 (both token blocks), rest of w1[ht0],
    remaining w1/w3 pairs, then w2 — the PE's first matmul fires ~4 us
    in and never starves after. w3[ht0] rides the otherwise-idle
    Activation queue.
  - output stores alternate between the Activation and SP queues so
    their data-waits cannot head-of-line-block anything; the last
    token-tile runs chain-per-dim-block so the final stores pipeline
    under compute instead of draining after it.

Stage 1 computes hT [H, T] = silu(w1 xT) * (w3 xT) with PSUM
[128h, 512t] x {2 token blocks}, 16 D-tile contraction; stage 2
computes out [T, D] with hT token-tiles stationary, PSUM [128t, 512d]
x {4 dim blocks}, 11 H-tile contraction. Measured on the 8-core axon
trn2: ~294 us/iteration (R=201/401 repeat-loop slope; sustained-load
clock throttling inflates longer horizons).
"""

from contextlib import ExitStack

import numpy as np

import concourse.bass as bass
import concourse.mybir as mybir
import concourse.tile as tile
from concourse import bacc
from concourse.bass import ts
from concourse.bass_utils import run_bass_kernel_spmd

F32 = mybir.dt.float32
BF16 = mybir.dt.bfloat16
NP_BF16 = mybir.dt.np(BF16)

N_TOKENS = 8192
DIM = 2048
HIDDEN = 1408
N_EXPERTS = 8
CAP = N_TOKENS // N_EXPERTS  # 1024 tokens per core
P = 128
KD = DIM // P  # 16 contraction tiles, stage 1
KH = HIDDEN // P  # 11 contraction tiles, stage 2
TB = 512  # token-block (stage-1 psum free dim)
DB = 512  # dim-block (stage-2 psum free dim)
N_TB = CAP // TB  # 2
N_DB = DIM // DB  # 4
N_TT = CAP // P  # 8 token tiles (stage-2 stationary)

_CACHED_NC = None


def _build_nc(repeat=1, body_reps=1, reuse=True):
    # repeat>1 wraps the whole body in a hardware loop; body_reps>1 emits the
    # body multiple times per iteration — both used only by the timing
    # harness to measure per-iteration device time past RPC overhead.
    # reuse=True emits back-to-back matmuls sharing one stationary tile
    # (experiment: does the backend skip redundant PE weight reloads?).
    nc = bacc.Bacc("TRN2", debug=False)
    xQ = nc.dram_tensor("xQ", [P, KD, CAP], BF16, kind="ExternalInput").ap()
    w1Q = nc.dram_tensor("w1Q", [KH, P, KD, P], BF16, kind="ExternalInput").ap()
    w3Q = nc.dram_tensor("w3Q", [KH, P, KD, P], BF16, kind="ExternalInput").ap()
    w2Q = nc.dram_tensor("w2Q", [N_DB, P, KH, DB], BF16, kind="ExternalInput").ap()
    out = nc.dram_tensor("out", [CAP, DIM], BF16, kind="ExternalOutput").ap()

    with tile.TileContext(nc) as tc, ExitStack() as ctx:
        xpool = ctx.enter_context(tc.tile_pool(name="xpool", bufs=1))
        hpool = ctx.enter_context(tc.tile_pool(name="hpool", bufs=1))
        wpool = ctx.enter_context(tc.tile_pool(name="wpool", bufs=KH))
        w2pool = ctx.enter_context(tc.tile_pool(name="w2pool", bufs=N_DB))
        tmppool = ctx.enter_context(tc.tile_pool(name="tmppool", bufs=3))
        opool = ctx.enter_context(tc.tile_pool(name="opool", bufs=4))
        pspool = ctx.enter_context(tc.tile_pool(name="pspool", bufs=2, space="PSUM"))

        if repeat > 1:
            ctx.enter_context(tc.For_i(0, repeat))

        emit = _emit_body_reuse if reuse else _emit_body
        for _rep in range(body_reps):
            emit(nc, _rep, xpool, hpool, wpool, w2pool, tmppool, opool,
                 pspool, xQ, w1Q, w3Q, w2Q, out)

    nc.compile()
    return nc


def _emit_body_reuse(nc, rep, xpool, hpool, wpool, w2pool, tmppool, opool,
                     pspool, xQ, w1Q, w3Q, w2Q, out):
    """Variant: consecutive matmuls share each stationary tile (both
    token-blocks per w-tile in stage 1; all four dim-blocks per h-tile in
    stage 2)."""
    x_sb = xpool.tile([P, KD, CAP], BF16, tag="x", name=f"x_sb_{rep}")
    h_sb = hpool.tile([P, KH, CAP], BF16, tag="h", name=f"h_sb_{rep}")

    w_t = {}

    def load_w13(which, ht, eng=None):
        Q = w1Q if which == 0 else w3Q
        t = wpool.tile(
            [P, KD, P], BF16, tag=f"w{which}", name=f"w{which}_{ht}_{rep}"
        )
        (eng or nc.sync).dma_start(t[:], Q[ht])
        w_t[(which, ht)] = t

    # both token blocks are consumed k-interleaved from the first chain, so
    # x streams k-major (full 1024-token rows per 2-k-tile block). The first
    # w1 tile goes in halves around the first x block so the PE's first
    # matmul fires ~4us in; w3[ht0] rides the otherwise-idle Act queue.
    w1_0 = wpool.tile([P, KD, P], BF16, tag="w0", name=f"w0_0_{rep}")
    w_t[(0, 0)] = w1_0
    nc.sync.dma_start(w1_0[:, 0 : KD // 2], w1Q[0][:, 0 : KD // 2])
    nc.sync.dma_start(x_sb[:, 0:2], xQ[:, 0:2])
    nc.sync.dma_start(w1_0[:, KD // 2 : KD], w1Q[0][:, KD // 2 : KD])
    load_w13(1, 0, eng=nc.scalar)
    for ko in range(2, KD, 2):
        nc.sync.dma_start(x_sb[:, ko : ko + 2], xQ[:, ko : ko + 2])
    for ht in range(1, KH):
        load_w13(0, ht)
        load_w13(1, ht)
    w2_t = []
    for db in range(N_DB):
        t = w2pool.tile([P, KH, DB], BF16, tag="w2", name=f"w2_{db}_{rep}")
        nc.sync.dma_start(t[:], w2Q[db])
        w2_t.append(t)

    # Stage 1: per (w, ht), one k-loop issuing both token-block matmuls
    # back-to-back on the same stationary w-tile.
    for ht in range(KH):
        acc = {}
        for which in range(2):
            psa = pspool.tile([P, TB], F32, tag=f"ps{which}a", bufs=1)
            psb = pspool.tile([P, TB], F32, tag=f"ps{which}b", bufs=1)
            for k in range(KD):
                nc.tensor.matmul(
                    psa[:], w_t[(which, ht)][:, k], x_sb[:, k, 0:TB],
                    start=(k == 0), stop=(k == KD - 1),
                )
                nc.tensor.matmul(
                    psb[:], w_t[(which, ht)][:, k], x_sb[:, k, TB : 2 * TB],
                    start=(k == 0), stop=(k == KD - 1),
                )
            acc[which] = (psa, psb)
        for tb in range(N_TB):
            sil = tmppool.tile([P, TB], F32, tag="sil")
            nc.scalar.activation(
                sil[:], acc[0][tb][:], mybir.ActivationFunctionType.Silu
            )
            nc.vector.tensor_mul(h_sb[:, ht, ts(tb, TB)], sil[:], acc[1][tb][:])

    # Stage 2: per tt, the four db chains share each h stationary tile
    # (k-outer, db-inner). The last tt instead runs chain-per-db so the
    # psum stops stagger and the final stores pipeline under compute.
    for tt in range(N_TT):
        pss = [
            pspool.tile([P, DB], F32, tag=f"ps2_{db}", bufs=1, name=f"ps2_{db}_{tt}")
            for db in range(N_DB)
        ]
        if tt < N_TT - 1:
            for k in range(KH):
                for db in range(N_DB):
                    nc.tensor.matmul(
                        pss[db][:], h_sb[:, k, ts(tt, P)], w2_t[db][:, k],
                        start=(k == 0), stop=(k == KH - 1),
                    )
            for db in range(N_DB):
                ot = opool.tile([P, DB], BF16, tag="ot")
                nc.vector.tensor_copy(ot[:], pss[db][:])
                eng = nc.scalar if db % 2 == 0 else nc.sync
                eng.dma_start(out[ts(tt, P), ts(db, DB)], ot[:])
        else:
            for db in range(N_DB):
                for k in range(KH):
                    nc.tensor.matmul(
                        pss[db][:], h_sb[:, k, ts(tt, P)], w2_t[db][:, k],
                        start=(k == 0), stop=(k == KH - 1),
                    )
                ot = opool.tile([P, DB], BF16, tag="ot")
                nc.vector.tensor_copy(ot[:], pss[db][:])
                eng = nc.scalar if db % 2 == 0 else nc.sync
                eng.dma_start(out[ts(tt, P), ts(db, DB)], ot[:])


def _emit_body(nc, rep, xpool, hpool, wpool, w2pool, tmppool, opool, pspool,
               xQ, w1Q, w3Q, w2Q, out):
    if True:  # keep indentation of the original body
        x_sb = xpool.tile([P, KD, CAP], BF16, tag="x", name=f"x_sb_{rep}")
        h_sb = hpool.tile([P, KH, CAP], BF16, tag="h", name=f"h_sb_{rep}")

        w_t = {}  # (which, ht) -> resident sbuf tile

        def load_w13(which, ht, eng=None):
            Q = w1Q if which == 0 else w3Q
            t = wpool.tile(
                [P, KD, P], BF16, tag=f"w{which}", name=f"w{which}_{ht}_{rep}"
            )
            (eng or nc.sync).dma_start(t[:], Q[ht])
            w_t[(which, ht)] = t

        # DMA issue order == arrival order == the order the PE needs data.
        # x goes in multi-k-tile blocks: big enough that the ~625ns HWDGE
        # issue overhead hides under the transfer, small enough to k-gate
        # chain 1. The first w1/x pieces are extra-fine so the PE's first
        # matmul fires ~3.5us in; w3[ht0] rides the otherwise-idle
        # Activation queue so it lands while chain 1 streams.
        w1_0 = wpool.tile([P, KD, P], BF16, tag="w0", name=f"w0_0_{rep}")
        w_t[(0, 0)] = w1_0
        nc.sync.dma_start(w1_0[:, 0 : KD // 2], w1Q[0][:, 0 : KD // 2])
        nc.sync.dma_start(x_sb[:, 0:2, 0:TB], xQ[:, 0:2, 0:TB])
        nc.sync.dma_start(w1_0[:, KD // 2 : KD], w1Q[0][:, KD // 2 : KD])
        load_w13(1, 0, eng=nc.scalar)
        for ko in (2, 4, 8, 12):  # x token-block 0, remaining k-tiles
            kc = 2 if ko == 2 else 4
            nc.sync.dma_start(x_sb[:, ko : ko + kc, 0:TB], xQ[:, ko : ko + kc, 0:TB])
        XC = 4
        for ht in range(1, KH):
            load_w13(0, ht)
            load_w13(1, ht)
        for ko in range(0, KD, XC):  # x token-block 1
            nc.sync.dma_start(
                x_sb[:, ko : ko + XC, TB : 2 * TB], xQ[:, ko : ko + XC, TB : 2 * TB]
            )
        w2_t = []
        for db in range(N_DB):  # stage-2 weights, fully resident
            t = w2pool.tile([P, KH, DB], BF16, tag="w2", name=f"w2_{db}_{rep}")
            nc.sync.dma_start(t[:], w2Q[db])
            w2_t.append(t)

        # Stage 1: hT = silu(w1 xT) * (w3 xT), psum [128h, 512t], 16 k-steps.
        for tb in range(N_TB):
            for ht in range(KH):
                ps1 = pspool.tile([P, TB], F32, tag="ps1", bufs=2)
                ps3 = pspool.tile([P, TB], F32, tag="ps3", bufs=2)
                for k in range(KD):
                    nc.tensor.matmul(
                        ps1[:], w_t[(0, ht)][:, k], x_sb[:, k, ts(tb, TB)],
                        start=(k == 0), stop=(k == KD - 1),
                    )
                for k in range(KD):
                    nc.tensor.matmul(
                        ps3[:], w_t[(1, ht)][:, k], x_sb[:, k, ts(tb, TB)],
                        start=(k == 0), stop=(k == KD - 1),
                    )
                sil = tmppool.tile([P, TB], F32, tag="sil")
                nc.scalar.activation(
                    sil[:], ps1[:], mybir.ActivationFunctionType.Silu
                )
                nc.vector.tensor_mul(h_sb[:, ht, ts(tb, TB)], sil[:], ps3[:])

        # Stage 2: out = hT.T @ w2.T, psum [128t, 512d], 11 k-steps.
        # Stores alternate between the Activation and SP queues to halve
        # per-queue backlog. The very last chain runs as two column-half
        # psum chains so its first store pipelines under its second half's
        # matmuls, shortening the end-of-kernel drain.
        for db in range(N_DB):
            for tt in range(N_TT):
                last = db == N_DB - 1 and tt == N_TT - 1
                if not last:
                    ps = pspool.tile([P, DB], F32, tag="ps2", bufs=2)
                    for k in range(KH):
                        nc.tensor.matmul(
                            ps[:], h_sb[:, k, ts(tt, P)], w2_t[db][:, k],
                            start=(k == 0), stop=(k == KH - 1),
                        )
                    ot = opool.tile([P, DB], BF16, tag="ot")
                    nc.vector.tensor_copy(ot[:], ps[:])
                    eng = nc.scalar if (db * N_TT + tt) % 2 == 0 else nc.sync
                    eng.dma_start(out[ts(tt, P), ts(db, DB)], ot[:])
                else:
                    half = DB // 2
                    for i, eng in enumerate((nc.scalar, nc.sync)):
                        ph = pspool.tile([P, half], F32, tag="ps2h", bufs=2)
                        for k in range(KH):
                            nc.tensor.matmul(
                                ph[:],
                                h_sb[:, k, ts(tt, P)],
                                w2_t[db][:, k, i * half : (i + 1) * half],
                                start=(k == 0), stop=(k == KH - 1),
                            )
                        oh = opool.tile([P, half], BF16, tag="oth")
                        nc.vector.tensor_copy(oh[:], ph[:])
                        eng.dma_start(
                            out[
                                ts(tt, P),
                                db * DB + i * half : db * DB + (i + 1) * half,
                            ],
                            oh[:],
                        )


def _get_nc():
    global _CACHED_NC
    if _CACHED_NC is None:
        _CACHED_NC = _build_nc()
    return _CACHED_NC


def _pack_inputs(x, w1, w2, w3, read_starts):
    """Per-core input dicts with DMA-optimal (partition-major) bf16 layouts."""
    in_maps = []
    for e in range(N_EXPERTS):
        s = int(read_starts[e])
        xe = x[s : s + CAP].astype(NP_BF16)  # [CAP, DIM]
        xQ = np.ascontiguousarray(xe.T.reshape(KD, P, CAP).transpose(1, 0, 2))
        w1Q = np.ascontiguousarray(
            w1[e].T.astype(NP_BF16).reshape(KD, P, KH, P).transpose(2, 1, 0, 3)
        )
        w3Q = np.ascontiguousarray(
            w3[e].T.astype(NP_BF16).reshape(KD, P, KH, P).transpose(2, 1, 0, 3)
        )
        w2Q = np.ascontiguousarray(
            w2[e].T.astype(NP_BF16).reshape(KH, P, N_DB, DB).transpose(2, 1, 0, 3)
        )
        in_maps.append({"xQ": xQ, "w1Q": w1Q, "w3Q": w3Q, "w2Q": w2Q})
    return in_maps


def kernel(x, num_tokens_per_expert, w1, w2, w3):
    x = np.ascontiguousarray(np.asarray(x, dtype=np.float32))
    w1 = np.asarray(w1, dtype=np.float32)
    w2 = np.asarray(w2, dtype=np.float32)
    w3 = np.asarray(w3, dtype=np.float32)
    counts = np.asarray(num_tokens_per_expert).astype(np.int64)

    offsets = np.cumsum(counts)
    starts = offsets - counts
    # jax.lax.dynamic_slice clamps the read start so the slice is in-bounds.
    read_starts = np.clip(starts, 0, N_TOKENS - CAP)

    in_maps = _pack_inputs(x, w1, w2, w3, read_starts)
    nc = _get_nc()
    res = run_bass_kernel_spmd(nc, in_maps, core_ids=list(range(N_EXPERTS)))
    ye = [
        np.asarray(res.results[e]["out"]).astype(np.float32)
        for e in range(N_EXPERTS)
    ]

    if np.all(counts == CAP):
        # balanced routing: per-expert tiles are disjoint and exactly cover x
        return np.concatenate(ye, axis=0)

    # general case: mask invalid slots, scatter-add to clipped positions
    y = np.zeros((N_TOKENS, DIM), np.float32)
    slot = np.arange(CAP)
    for e in range(N_EXPERTS):
        valid = slot < counts[e]
        pos = np.clip(starts[e] + slot, 0, N_TOKENS - 1)
        np.add.at(y, pos, np.where(valid[:, None], ye[e], 0.0))
    return y
